# revision 1
# baseline (speedup 1.0000x reference)
"""NeRF render kernel for 8 Trainium2 NeuronCores.

Data-parallel over rays: core k handles rays [2048*k, 2048*(k+1)).
Per core: positional encoding + 3-layer MLP (39->256->256->4) over
131072 points in feature-major layout (features on partitions, points
on the free dim), then alpha compositing via triangular-matrix matmul
cumulative sums.

Point permutation inside a core: t_rand rows are loaded as
[128 partitions = ray-group i (rays 16i..16i+15), 1024 = (k, s)] and
PE-transposed per 128-column chunk k0 so that partitions become
q = rp*64 + s (rp = ray parity) and columns J = 128*k0 + i denote the
ray pair (16i + 2*k0, 16i + 2*k0 + 1).  All downstream tiles keep that
column order; the host unscatters at the end.
"""

import sys
import numpy as np

sys.path.insert(0, "/opt/trn_rl_repo")

S = 64
L = 6
NCORES = 8
B = 16384
BC = B // NCORES          # rays per core
NP = BC * S               # points per core
NBLK = NP // 128          # 1024 ray-pair blocks
NGRP = 8                  # groups of 128 blocks
TPG = NBLK // NGRP // 4   # tiles (of 4 blocks / 512 points) per group
NEAR, FAR = 2.0, 6.0
DELTA = (FAR - NEAR) / S
PI = float(np.pi)
TWO_PI = float(2.0 * np.pi)
INV2PI = float(np.float32(1.0 / (2.0 * np.pi)))
MAGIC = 12582912.0  # 1.5 * 2**23: float32 round-to-int trick
C1 = float(np.float32(2.0 * np.pi))
C2 = float(2.0 * np.pi - np.float64(np.float32(2.0 * np.pi)))

_CACHE = {}
PROFILE = False  # test harness sets True to collect an NTFF trace


def _split_waits(nc, mybir):
    """TRN2 allows one sem wait per instruction (two for EventSemaphore);
    this walrus build rejects over-limit instructions, so move excess waits
    onto chained NOPs on the same engine just before the instruction."""
    ctr = 0
    for fn in nc.m.functions:
        for bb in fn.blocks:
            changed = False
            out = []
            for inst in bb.instructions:
                si = inst.sync_info
                cap = 2 if isinstance(inst, mybir.InstEventSemaphore) else 1
                if si is not None and si.on_wait and len(si.on_wait) > cap:
                    waits = list(si.on_wait)
                    for w in waits[:-cap]:
                        nop = mybir.InstNoOp(
                            name=f"wsplit-{ctr}", ins=[], outs=[]
                        )
                        ctr += 1
                        nop.engine = inst.engine
                        nop.sync_info = mybir.SyncInfo(on_wait=[w], on_update=[])
                        nc.register_instruction(nop)
                        out.append(nop)
                    si.on_wait = waits[-cap:]
                    changed = True
                out.append(inst)
            if changed:
                bb.instructions = out
    return ctr


def _build():
    import concourse.bass as bass
    import concourse.mybir as mybir
    import concourse.tile as tile

    dt = mybir.dt
    AF = mybir.ActivationFunctionType
    OP = mybir.AluOpType
    F32 = dt.float32
    F32R = dt.float32r
    F16 = dt.float16

    nc = bass.Bass()

    # ---- DRAM I/O ----
    tnat_d = nc.dram_tensor("tnat", [128, 1024], F32, kind="ExternalInput")
    aexp_d = nc.dram_tensor("aexp", [3, 128, 1024], F32, kind="ExternalInput")
    bexp_d = nc.dram_tensor("bexp", [3, 128, 1024], F32, kind="ExternalInput")
    w0_d = nc.dram_tensor("w0p", [39, 256], F16, kind="ExternalInput")
    w1_d = nc.dram_tensor("w1", [256, 256], F16, kind="ExternalInput")
    w2_d = nc.dram_tensor("w2h", [128, 8], F16, kind="ExternalInput")
    b0_d = nc.dram_tensor("b0t", [128, 2], F32, kind="ExternalInput")
    b1_d = nc.dram_tensor("b1t", [128, 2], F32, kind="ExternalInput")
    b2_d = nc.dram_tensor("b2t", [128, 4], F32, kind="ExternalInput")
    zcpp_d = nc.dram_tensor("zcpp", [128, 1], F32, kind="ExternalInput")
    ltri_d = nc.dram_tensor("ltri", [128, 256], F32, kind="ExternalInput")
    sel2_d = nc.dram_tensor("sel2", [128, 2], F32R, kind="ExternalInput")
    ident_d = nc.dram_tensor("ident", [128, 128], F32R, kind="ExternalInput")
    identh_d = nc.dram_tensor("identh", [128, 128], F16, kind="ExternalInput")
    out_d = nc.dram_tensor("out", [NGRP, 2, 384], F32, kind="ExternalOutput")

    def r(ap):
        return ap.bitcast(F32R)

    with tile.TileContext(nc) as tc:
        with (
            tc.tile_pool(name="consts", bufs=1) as cpool,
            tc.tile_pool(name="tall", bufs=1) as tpool,
            tc.tile_pool(name="o2", bufs=NGRP) as o2pool,
        ):
            # ---- load constants / weights ----
            tnat = cpool.tile([128, 1024], F32, tag="tnat")
            nc.sync.dma_start(tnat[:], tnat_d[:])
            w0s = cpool.tile([39, 256], F16, tag="w0s")
            nc.sync.dma_start(w0s[:], w0_d[:])
            w1s0 = cpool.tile([128, 256], F16, tag="w1s0")
            nc.sync.dma_start(w1s0[:], w1_d[0:128, :])
            w1s1 = cpool.tile([128, 256], F16, tag="w1s1")
            nc.sync.dma_start(w1s1[:], w1_d[128:256, :])
            w2s = cpool.tile([128, 8], F16, tag="w2s")
            nc.sync.dma_start(w2s[:], w2_d[:])
            b0t = cpool.tile([128, 2], F32, tag="b0t")
            nc.sync.dma_start(b0t[:], b0_d[:])
            b1t = cpool.tile([128, 2], F32, tag="b1t")
            nc.sync.dma_start(b1t[:], b1_d[:])
            b2t = cpool.tile([128, 4], F32, tag="b2t")
            nc.sync.dma_start(b2t[:], b2_d[:])
            zcpp = cpool.tile([128, 1], F32, tag="zcpp")
            nc.sync.dma_start(zcpp[:], zcpp_d[:])
            ltri = cpool.tile([128, 256], F32, tag="ltri")
            nc.sync.dma_start(ltri[:], ltri_d[:])
            sel2 = cpool.tile([128, 2], F32R, tag="sel2")
            nc.sync.dma_start(sel2[:], sel2_d[:])
            ident = cpool.tile([128, 128], F32R, tag="ident")
            nc.sync.dma_start(ident[:], ident_d[:])
            identh = cpool.tile([128, 128], F16, tag="identh")
            nc.sync.dma_start(identh[:], identh_d[:])
            zerot = cpool.tile([128, 1], F32, tag="zerot")
            nc.vector.memset(zerot[:], 0.0)

            # ---- phase A: t transpose, z, pts (tall/block layout) ----
            zt = tpool.tile([128, 1024], F32, tag="zt")
            pts3 = tpool.tile([128, 3072], F32, tag="pts3")
            pts3r = tpool.tile([128, 3072], F32, tag="pts3r")
            with (
                tc.tile_pool(name="apool", bufs=1) as apool,
                tc.tile_pool(name="ttp", bufs=2, space="PSUM") as ttp_pool,
            ):
                for k0 in range(8):
                    ttp = ttp_pool.tile([128, 128], F32, tag="ttp")
                    nc.tensor.transpose(
                        ttp[:], tnat[:, 128 * k0 : 128 * (k0 + 1)], ident[:].bitcast(F32)
                    )
                    # z = delta * t + zc[s]
                    nc.vector.tensor_scalar(
                        zt[:, 128 * k0 : 128 * (k0 + 1)],
                        ttp[:],
                        DELTA,
                        zcpp[:, 0:1],
                        op0=OP.mult,
                        op1=OP.add,
                    )
                aexs = []
                bexs = []
                for c in range(3):
                    ae = apool.tile([128, 1024], F32, tag=f"aex{c}")
                    nc.sync.dma_start(ae[:], aexp_d[c])
                    be = apool.tile([128, 1024], F32, tag=f"bex{c}")
                    nc.sync.dma_start(be[:], bexp_d[c])
                    aexs.append(ae)
                    bexs.append(be)
                for c in range(3):
                    pv = pts3.rearrange("p (j c) -> p c j", c=3)[:, c, :]
                    nc.vector.tensor_tensor(pv, zt[:], bexs[c][:], op=OP.mult)
                    nc.vector.tensor_tensor(pv, pv, aexs[c][:], op=OP.add)
                # base range reduction: pts3r = pts3 - 2pi*round(pts3/2pi)
                kt = apool.tile([128, 3072], F32, tag="kt")
                nc.vector.tensor_scalar(
                    kt[:], pts3[:], INV2PI, MAGIC, op0=OP.mult, op1=OP.add
                )
                nc.vector.tensor_scalar(
                    kt[:], kt[:], MAGIC, None, op0=OP.subtract
                )
                nc.vector.scalar_tensor_tensor(
                    pts3r[:], kt[:], -C1, pts3[:], op0=OP.mult, op1=OP.add
                )
                nc.vector.scalar_tensor_tensor(
                    pts3r[:], kt[:], -C2, pts3r[:], op0=OP.mult, op1=OP.add
                )
                nc.vector.tensor_scalar(
                    pts3r[:], pts3r[:], PI, -PI, op0=OP.min, op1=OP.max
                )

            # ---- phase B: features + MLP per 512-point tile ----
            with (
                tc.tile_pool(name="scr", bufs=2) as scrpool,
                tc.tile_pool(name="kpool", bufs=2) as kpool,
                tc.tile_pool(name="sfp", bufs=2) as sfppool,
                tc.tile_pool(name="fs", bufs=4) as fspool,
                tc.tile_pool(name="tpP", bufs=2, space="PSUM") as tp_pool,
                tc.tile_pool(name="h0s", bufs=4) as h0spool,
                tc.tile_pool(name="h1s", bufs=4) as h1spool,
                tc.tile_pool(name="h0P", bufs=2, space="PSUM") as h0_pool,
                tc.tile_pool(name="h1P", bufs=2, space="PSUM") as h1_pool,
                tc.tile_pool(name="oP", bufs=2, space="PSUM") as o_pool,
            ):
                HB = 64  # blocks per half-group
                for g in range(NGRP):
                    og = o_pool.tile([128, 512], F32, tag="og")
                    for half in range(2):
                        hg = 2 * g + half
                        # chain scratch: args per block j (f-cols: 3l+c
                        # sin-arg, 18+3l+c cos-arg), doubling + wrap chain
                        # from the base-reduced pts.
                        sa = scrpool.tile([128, HB * 36], F32, tag="sa")
                        sav = sa.rearrange("p (j f) -> p j f", j=HB)
                        p3v = pts3.rearrange("p (j c) -> p j c", c=3)[
                            :, HB * hg : HB * (hg + 1), :
                        ]
                        p3rv = pts3r.rearrange("p (j c) -> p j c", c=3)[
                            :, HB * hg : HB * (hg + 1), :
                        ]
                        nc.vector.tensor_copy(sav[:, :, 0:3], p3rv)
                        # sin l: r_l = 2 r_{l-1} - 2pi*round(2 r_{l-1}/2pi)
                        for l in range(1, L):
                            prev = sav[:, :, 3 * (l - 1) : 3 * l]
                            cur = sav[:, :, 3 * l : 3 * l + 3]
                            kb = kpool.tile([128, HB * 3], F32, tag="kb")
                            kbv = kb.rearrange("p (j c) -> p j c", c=3)
                            nc.vector.tensor_scalar(
                                kbv, prev, 2.0 * INV2PI, MAGIC,
                                op0=OP.mult, op1=OP.add,
                            )
                            nc.vector.tensor_scalar(
                                kbv, kbv, MAGIC, C1,
                                op0=OP.subtract, op1=OP.mult,
                            )
                            nc.vector.scalar_tensor_tensor(
                                cur, prev, 2.0, kbv,
                                op0=OP.mult, op1=OP.subtract,
                            )
                        # cos l: c_l = (r_l + pi/2) - 2pi*[r_l > pi/2]
                        for l in range(L):
                            rl = sav[:, :, 3 * l : 3 * l + 3]
                            cl = sav[:, :, 18 + 3 * l : 21 + 3 * l]
                            kb = kpool.tile([128, HB * 3], F32, tag="kb")
                            kbv = kb.rearrange("p (j c) -> p j c", c=3)
                            nc.vector.tensor_scalar(
                                kbv, rl, PI / 2, C1, op0=OP.is_gt, op1=OP.mult
                            )
                            nc.vector.scalar_tensor_tensor(
                                cl, rl, PI / 2, kbv, op0=OP.add, op1=OP.subtract
                            )
                        # trig + raw pts into the fp16 staging tile (tall)
                        sf = sfppool.tile([128, HB * 39], F16, tag="sf")
                        sfv = sf.rearrange("p (j f) -> p j f", j=HB)
                        nc.scalar.activation(
                            sfv[:, :, 0:36], sav[:, :, 0:36], AF.Sin
                        )
                        nc.vector.tensor_copy(sfv[:, :, 36:39], p3v)
                        for itl in range(HB // 4):
                            # PE-transpose 4 blocks -> feature-major fp16
                            tp = tp_pool.tile([39, 512], F16, tag="tp")
                            for jp in range(4):
                                jj = 4 * itl + jp
                                nc.tensor.transpose(
                                    tp[:, 128 * jp : 128 * (jp + 1)],
                                    sf[:, 39 * jj : 39 * (jj + 1)],
                                    identh[:],
                                )
                            fs = fspool.tile([39, 512], F16, tag="fs")
                            nc.scalar.activation(
                                fs[:, 0:256], tp[:, 0:256], AF.Copy
                            )
                            nc.vector.tensor_copy(
                                fs[:, 256:512], tp[:, 256:512]
                            )
                            # L0
                            h0ss = []
                            for h in range(2):
                                h0p = h0_pool.tile([128, 512], F32, tag="h0p")
                                nc.tensor.matmul(
                                    h0p[:],
                                    w0s[:, 128 * h : 128 * (h + 1)],
                                    fs[:],
                                )
                                h0s = h0spool.tile([128, 512], F16, tag="h0s")
                                nc.scalar.activation(
                                    h0s[:], h0p[:], AF.Relu,
                                    bias=b0t[:, h : h + 1],
                                )
                                h0ss.append(h0s)
                            # L1
                            h1ss = []
                            for h in range(2):
                                h1p = h1_pool.tile([128, 512], F32, tag="h1p")
                                nc.tensor.matmul(
                                    h1p[:],
                                    w1s0[:, 128 * h : 128 * (h + 1)],
                                    h0ss[0][:],
                                    start=True,
                                    stop=False,
                                )
                                nc.tensor.matmul(
                                    h1p[:],
                                    w1s1[:, 128 * h : 128 * (h + 1)],
                                    h0ss[1][:],
                                    start=False,
                                    stop=True,
                                )
                                if h == 0:
                                    h1s = h1spool.tile(
                                        [128, 512], F16, tag="h1s"
                                    )
                                    nc.scalar.activation(
                                        h1s[:], h1p[:], AF.Relu,
                                        bias=b1t[:, h : h + 1],
                                    )
                                else:
                                    h1s = h1spool.tile(
                                        [128, 512], F16, tag="h1s"
                                    )
                                    nc.vector.tensor_scalar(
                                        h1s[:],
                                        h1p[:],
                                        b1t[:, h : h + 1],
                                        0.0,
                                        op0=OP.add,
                                        op1=OP.max,
                                    )
                                h1ss.append(h1s)
                            # L2: activations stationary, W2 moving
                            for jp in range(4):
                                jj = 64 * half + 4 * itl + jp
                                nc.tensor.matmul(
                                    og[:, 4 * jj : 4 * (jj + 1)],
                                    h1ss[0][:, 128 * jp : 128 * (jp + 1)],
                                    w2s[:, 0:4],
                                    start=True,
                                    stop=False,
                                )
                                nc.tensor.matmul(
                                    og[:, 4 * jj : 4 * (jj + 1)],
                                    h1ss[1][:, 128 * jp : 128 * (jp + 1)],
                                    w2s[:, 4:8],
                                    start=False,
                                    stop=True,
                                )
                    # ---- drain O psum -> O2 sbuf (bias, relu on sigma) ----
                    o2 = o2pool.tile([128, 512], F32, tag="o2")
                    orgb = og.rearrange("p (j c) -> p j c", c=4)[:, :, 0:3]
                    o2rgb = o2.rearrange("p (j c) -> p j c", c=4)[:, :, 0:3]
                    brgb = b2t[:, 0:3].unsqueeze(1).broadcast_to([128, 128, 3])
                    nc.vector.tensor_tensor(o2rgb, orgb, brgb, op=OP.add)
                    osig = og.rearrange("p (j c) -> p j c", c=4)[:, :, 3]
                    o2sig = o2.rearrange("p (j c) -> p j c", c=4)[:, :, 3]
                    zbc = zerot[:, 0:1].broadcast_to([128, 128])
                    nc.vector.scalar_tensor_tensor(
                        o2sig, osig, b2t[:, 3:4], zbc, op0=OP.add, op1=OP.max
                    )
                    if g == 0:
                        o2s = []
                    o2s.append(o2)

            # ---- phase C: compositing (exp table) ----
            tc.no_sync_barrier()
            with (
                tc.tile_pool(name="cS", bufs=2) as cspool,
                tc.tile_pool(name="cP", bufs=2, space="PSUM") as cppool,
            ):
                for g in range(NGRP):
                    o2 = o2s[g]
                    o2v = o2.rearrange("p (j c) -> p j c", c=4)
                    # sigmoid via exp + reciprocal
                    e = cspool.tile([128, 384], F32, tag="e")
                    nc.scalar.activation(
                        e.rearrange("p (j c) -> p j c", c=3),
                        o2v[:, :, 0:3],
                        AF.Exp,
                        scale=-1.0,
                    )
                    nc.vector.tensor_scalar(e[:], e[:], 1.0, None, op0=OP.add)
                    nc.vector.reciprocal(e[:], e[:])
                    # scans: exclusive & inclusive cumsum of sigma over s
                    scp = cppool.tile([128, 256], F32, tag="scp")
                    sig = o2v[:, :, 3]
                    nc.tensor.matmul(scp[:, 0:128], ltri[:, 0:128], sig)
                    nc.tensor.matmul(scp[:, 128:256], ltri[:, 128:256], sig)
                    texin = cspool.tile([128, 256], F32, tag="texin")
                    nc.scalar.activation(texin[:], scp[:], AF.Exp, scale=-DELTA)
                    wt = cspool.tile([128, 128], F32, tag="wt")
                    nc.vector.tensor_tensor(
                        wt[:], texin[:, 0:128], texin[:, 128:256], op=OP.subtract
                    )
                    wr = cspool.tile([128, 384], F32R, tag="wr")
                    nc.vector.tensor_tensor(
                        wr.rearrange("p (j c) -> p j c", c=3),
                        e.rearrange("p (j c) -> p j c", c=3),
                        wt.unsqueeze(2).broadcast_to([128, 128, 3]),
                        op=OP.mult,
                    )
                    rp_ = cppool.tile([2, 384], F32, tag="rp")
                    nc.tensor.matmul(rp_[:], sel2[:], wr[:])
                    outs = cspool.tile([2, 384], F32, tag="outs")
                    nc.vector.tensor_copy(outs[:], rp_[:])
                    nc.sync.dma_start(out_d[g], outs[:])

    _split_waits(nc, mybir)
    return nc


def _host_prep(origins, directions, t_rand, W0, b0, W1, b1, W2, b2):
    """Build per-core input maps (all numpy, cheap)."""
    f32 = np.float32
    # F-row order: rows 3l+c = sin freq l coord c; 18+3l+c = cos; 36..38 pts
    perm = np.zeros(39, np.int64)
    perm[36:39] = (0, 1, 2)
    for l in range(L):
        for c in range(3):
            perm[3 * l + c] = 3 + 6 * l + c
            perm[18 + 3 * l + c] = 3 + 6 * l + 3 + c
    w0p = np.ascontiguousarray(W0[perm]).astype(np.float16)

    w2h = np.empty((128, 8), np.float16)
    w2h[:, 0:4] = W2[0:128].astype(np.float16)
    w2h[:, 4:8] = W2[128:256].astype(np.float16)
    b0t = np.ascontiguousarray(b0.reshape(2, 128).T).astype(f32)
    b1t = np.ascontiguousarray(b1.reshape(2, 128).T).astype(f32)
    b2t = np.broadcast_to(b2.astype(f32), (128, 4)).copy()

    q = np.arange(128)
    rp = q // 64
    s = q % 64
    zcpp = (NEAR + DELTA * s).astype(f32).reshape(128, 1).copy()

    # ltri: cols 0..127 exclusive, 128..255 inclusive
    # ltri[k=(rp',j), m=(rp,s)] = (rp'==rp) & (j < s)  /  (j <= s)
    kk = q
    krp = kk // 64
    kj = kk % 64
    same = (krp[:, None] == rp[None, :])
    ltri = np.zeros((128, 256), f32)
    ltri[:, 0:128] = (same & (kj[:, None] < s[None, :])).astype(f32)
    ltri[:, 128:256] = (same & (kj[:, None] <= s[None, :])).astype(f32)
    sel2 = (krp[:, None] == np.arange(2)[None, :]).astype(f32)
    ident = np.eye(128, dtype=f32)
    identh = np.eye(128, dtype=np.float16)

    # ray_of[J, rp] = 16*(J%128) + 2*(J//128) + rp
    J = np.arange(NBLK)
    ray_of = (16 * (J % 128))[:, None] + (2 * (J // 128))[:, None] + np.arange(2)[None, :]

    in_maps = []
    for core in range(NCORES):
        o = origins[core * BC : (core + 1) * BC].astype(f32)
        d = directions[core * BC : (core + 1) * BC].astype(f32)
        t = t_rand[core * BC : (core + 1) * BC].astype(f32)
        tnat = np.ascontiguousarray(t.reshape(128, 1024))
        # aexp[c, q, J] = o[ray_of[J, rp(q)], c]
        rays_qJ = ray_of[:, :].T[rp]  # [128, NBLK] -> rays_qJ[q, J] = ray_of[J, rp[q]]
        aexp = np.ascontiguousarray(o[rays_qJ].transpose(2, 0, 1))
        bexp = np.ascontiguousarray(d[rays_qJ].transpose(2, 0, 1))
        in_maps.append(
            {
                "tnat": tnat,
                "aexp": aexp,
                "bexp": bexp,
                "w0p": w0p,
                "w1": W1.astype(np.float16),
                "w2h": w2h,
                "b0t": b0t,
                "b1t": b1t,
                "b2t": b2t,
                "zcpp": zcpp,
                "ltri": ltri,
                "sel2": sel2,
                "ident": ident,
                "identh": identh,
            }
        )
    return in_maps, ray_of


def kernel(origins, directions, t_rand, W0, b0, W1, b1, W2, b2, near, far,
           **kw):
    assert int(near) == 2 and int(far) == 6
    from concourse.bass_utils import run_bass_kernel_spmd

    if "nc" not in _CACHE:
        _CACHE["nc"] = _build()
    nc = _CACHE["nc"]

    in_maps, ray_of = _host_prep(
        np.asarray(origins), np.asarray(directions), np.asarray(t_rand),
        np.asarray(W0), np.asarray(b0), np.asarray(W1), np.asarray(b1),
        np.asarray(W2), np.asarray(b2),
    )
    res = run_bass_kernel_spmd(
        nc, in_maps, core_ids=list(range(NCORES)), trace=PROFILE
    )
    _CACHE["last_results"] = res
    out = np.empty((B, 3), np.float32)
    for core in range(NCORES):
        oc = res.results[core]["out"].reshape(NGRP, 2, 128, 3)
        # group g holds blocks J = 128*g + i ; ray = 16*i + 2*g + rp
        for g in range(NGRP):
            for rpp in range(2):
                rays = core * BC + 16 * np.arange(128) + 2 * g + rpp
                out[rays] = oc[g, rpp]
    return out

